# revision 19
# baseline (speedup 1.0000x reference)
"""Multi-head attention (B=2, N=2048, D=1024, H=16, dh=64) on 8 TRN2 cores.

Sharding: (batch x head-group) -- core c handles batch c//4 and heads
[4*(c%4), 4*(c%4)+4) (256 local dims). Each core computes its heads'
Q/K/V projections, attention, and a partial output projection; the host
sums 4 partials per batch and adds bo. Halves per-core input DMA vs
head-only sharding (each core loads only its batch's activations).

Per-core design notes (PE kept continuously busy to hold the HAM clock
gate at 2.4 GHz; exp on the scalar engine is the second-longest stream
and runs as [128,1024] tiles to amortize per-instruction overhead):
  - X^T [D, N] supplied by host; Q^T/K^T computed with W stationary
    ([dl, tok], dl on partitions; bias via per-partition tensor_scalar).
  - V computed directly as [tok, dl] (x^T chunks stationary) into
    vaug = [V_h | ones] per head; ones column yields the softmax
    denominator Z for free during ctx accumulation.
  - scoresT[k, q] per (head, kt): two 512-col matmuls into a 2-bank
    [128,1024] f32 PSUM tile; one exp per tile.
  - ctxT[dv, q] accumulated over kt in [128,512] PSUM chunks; chunks
    are copied (unnormalized) to SBUF right away so the PSUM bank ring
    never stalls the PE at block boundaries.
  - normalization: Z row -> DRAM roundtrip partition-broadcast ->
    reciprocal_approx_fast -> multiply (engines cannot replicate
    across partitions; DMA can). Emitted as deferred stages inside the
    next block so latency hides behind matmuls.
  - output projection: out[q, od] = stack^T @ Wo in two K=128 pieces
    (dl-tiles), bf16 partials to HBM; host sums in f32.
  - DMA queues: sync = bulk loads, scalar = latency-critical Z/stack
    moves, gpsimd = output stores.
"""

import numpy as np
import ml_dtypes
from contextlib import ExitStack

import concourse.bass as bass
import concourse.tile as tile
from concourse import bacc, mybir
from concourse.bass import ts, ds
from concourse.bass_utils import run_bass_kernel_spmd

BF16 = mybir.dt.bfloat16
F32 = mybir.dt.float32

B = 2
N = 2048          # tokens per batch
D = 1024          # model dim
NCORES = 8
HLOC = 4          # heads per core
DLOC = 256        # local dims per core (4 heads x 64)
DH = 64
NKT = N // 128    # 16 k-tiles of 128
NDCH = D // 128   # 8 d-chunks
NQH = 2           # q halves of 1024
QH = 1024
NBLK = HLOC * NQH # 8 attention blocks per core


def _build_program():
    nc = bacc.Bacc("TRN2", target_bir_lowering=False, debug=False)

    xT = {}
    w = {}
    for t in ("q", "k", "v"):
        xT[t] = nc.dram_tensor(f"x{t}T", [D, N], BF16, kind="ExternalInput").ap()
        w[t] = nc.dram_tensor(f"w{t}", [D, DLOC], BF16, kind="ExternalInput").ap()
    bias = {}
    for t in ("q", "k"):
        bias[t] = nc.dram_tensor(f"b{t}", [DLOC, 1], F32, kind="ExternalInput").ap()
    bvd = nc.dram_tensor("bv", [1, DLOC], F32, kind="ExternalInput").ap()
    wo = nc.dram_tensor("wo", [DLOC, D], BF16, kind="ExternalInput").ap()
    outp = nc.dram_tensor("outp", [N, D], BF16, kind="ExternalOutput").ap()
    zscr = nc.dram_tensor("zscr", [NBLK, QH], F32).ap()

    with ExitStack() as ctx:
        tc = ctx.enter_context(tile.TileContext(nc))

        const = ctx.enter_context(tc.tile_pool(name="const", bufs=1))
        xqkp = ctx.enter_context(tc.tile_pool(name="xqk", bufs=32))
        xvp = ctx.enter_context(tc.tile_pool(name="xv", bufs=16))
        qkp = ctx.enter_context(tc.tile_pool(name="qk", bufs=1))
        vaugp = ctx.enter_context(tc.tile_pool(name="vaug", bufs=1))
        cxp = ctx.enter_context(tc.tile_pool(name="cxu", bufs=3))
        zp = ctx.enter_context(tc.tile_pool(name="zsb", bufs=2))
        bcp = ctx.enter_context(tc.tile_pool(name="bc", bufs=2))
        recp = ctx.enter_context(tc.tile_pool(name="rec", bufs=2))
        stackp = ctx.enter_context(tc.tile_pool(name="stack", bufs=2))
        hbp = ctx.enter_context(tc.tile_pool(name="hb", bufs=2))
        obp = ctx.enter_context(tc.tile_pool(name="ob", bufs=4))

        pp_s = ctx.enter_context(tc.tile_pool(name="pp_s", bufs=2, space="PSUM"))
        pp_c = ctx.enter_context(tc.tile_pool(name="pp_c", bufs=2, space="PSUM"))
        pp_pr = ctx.enter_context(tc.tile_pool(name="pp_pr", bufs=1, space="PSUM"))
        pp_sh = ctx.enter_context(tc.tile_pool(name="pp_sh", bufs=1, space="PSUM"))

        # ---- constants + x loads, ordered for just-in-time arrival:
        # q-h0, k-h0 (upfront proj) | k-h1 | xv-h0 | q-h1 | xv-h1 ----
        w_sb = {}
        b_sb = {}
        xtiles = {}

        def load_w(t):
            w_sb[t] = const.tile([128, NDCH, DLOC], BF16, tag=f"w{t}", name=f"w{t}sb")
            nc.sync.dma_start(out=w_sb[t], in_=w[t].rearrange("(c p) m -> p c m", p=128))

        def load_x(t, hf):
            for c in range(NDCH):
                xt_ = xqkp.tile([128, QH], BF16, tag="x", name="xt")
                nc.sync.dma_start(out=xt_, in_=xT[t][ts(c, 128), ts(hf, QH)])
                xtiles[(t, c, hf)] = xt_

        def load_xv(hf):
            for c in range(NDCH):
                xt_ = xvp.tile([128, QH], BF16, tag="xv", name="xvt")
                nc.sync.dma_start(out=xt_, in_=xT["v"][ts(c, 128), ts(hf, QH)])
                xtiles[("v", c, hf)] = xt_

        for t in ("q", "k"):
            load_w(t)
            b_sb[t] = const.tile([128, 2, 1], F32, tag=f"b{t}", name=f"b{t}sb")
            nc.sync.dma_start(out=b_sb[t],
                              in_=bias[t].rearrange("(t p) o -> p t o", p=128))
            load_x(t, 0)
        load_x("k", 1)
        load_w("v")
        load_xv(0)
        load_x("q", 1)
        load_xv(1)
        bvbc = const.tile([128, HLOC, DH], F32, tag="bvbc")
        seg = bvd[0, :]
        nc.sync.dma_start(
            out=bvbc,
            in_=bass.AP(tensor=seg.tensor, offset=seg.offset,
                        ap=[[0, 128]] + list(seg.ap)))
        wo_sb = const.tile([128, 2, D], BF16, tag="wo")
        nc.sync.dma_start(out=wo_sb, in_=wo.rearrange("(t p) d -> p t d", p=128))

        # warm the exp table load off the critical path
        warm = const.tile([128, 1], BF16, tag="warm")
        nc.scalar.activation(warm, b_sb["q"][:, 0, :], mybir.ActivationFunctionType.Exp)

        # ---- Q^T / K^T projections ([dl-tile, tok], W stationary) ----
        # Upfront: only the first token-half of q and k (enough to start
        # attention block 0); the rest are emitted as paced sub-chunk
        # fillers inside the attention stream.
        qt_sb = qkp.tile([128, 2, N], BF16, tag="qt")
        kt_sb = qkp.tile([128, 2, N], BF16, tag="kt")
        dest = {"q": qt_sb, "k": kt_sb}
        for t in ("q", "k"):
            psD = [pp_s.tile([128, QH], F32, tag="s", name=f"psD{dl}")
                   for dl in range(2)]
            for c in range(NDCH):
                for dl in range(2):
                    for sh in range(2):
                        nc.tensor.matmul(
                            psD[dl][:, ts(sh, 512)],
                            lhsT=w_sb[t][:, c, ds(dl * 128, 128)],
                            rhs=xtiles[(t, c, 0)][:, ts(sh, 512)],
                            start=(c == 0),
                            stop=(c == NDCH - 1),
                        )
            for dl in range(2):
                for sh in range(2):
                    nc.vector.tensor_scalar_add(
                        dest[t][:, dl, ts(sh, 512)],
                        psD[dl][:, ts(sh, 512)], b_sb[t][:, dl, :])

        def proj_subchunk(t, dl, sh):
            # one [128,512] psum chunk: token-half 1, given dl-tile and
            # 512-token sub-slice; 8 accumulating matmuls + bias
            psP = pp_pr.tile([128, 512], F32, tag="pr", name="psP")
            for c in range(NDCH):
                nc.tensor.matmul(
                    psP,
                    lhsT=w_sb[t][:, c, ds(dl * 128, 128)],
                    rhs=xtiles[(t, c, 1)][:, ts(sh, 512)],
                    start=(c == 0),
                    stop=(c == NDCH - 1),
                )
            nc.vector.tensor_scalar_add(
                dest[t][:, dl, ds(QH + sh * 512, 512)], psP, b_sb[t][:, dl, :])

        pend_pe = [lambda t=t, dl=dl, sh=sh: proj_subchunk(t, dl, sh)
                   for (t, dl, sh) in (("k", 0, 0), ("k", 0, 1),
                                       ("q", 0, 0), ("q", 0, 1),
                                       ("k", 1, 0), ("k", 1, 1),
                                       ("q", 1, 0), ("q", 1, 1))]

        # ---- attention ----
        # vaug: [tok-part, kt, h, 65] = [V_h | ones]
        vaug = vaugp.tile([128, NKT, HLOC, 65], BF16, tag="vaug")
        nc.vector.memset(vaug[:, :, :, 64:65], 1.0)

        stacks = {}      # qh -> stack tile [128, 2, QH]
        pend = []        # deferred work stages (one popped per drain slot)
        pend_wo = []     # deferred wo od-pairs
        wo_ready = set() # q-halves whose stack is fully written

        def emit_scores(h, qh, kt):
            lo = (h % 2) * 64
            t = h // 2
            psS = pp_s.tile([128, QH], F32, tag="s", name="psS")
            for half in range(2):
                nc.tensor.matmul(
                    psS[:, ts(half, 512)],
                    lhsT=kt_sb[ds(lo, 64), t, ts(kt, 128)],
                    rhs=qt_sb[ds(lo, 64), t, ds(qh * QH + half * 512, 512)],
                    start=True, stop=True,
                )
            # e tiles reuse the (dead) x half-tile ring: a deep buffer that
            # lets the ctx stream lag the scores stream by CTX_LAG k-tiles.
            e = xqkp.tile([128, QH], BF16, tag="x", name="e_t")
            nc.scalar.activation(e, psS, mybir.ActivationFunctionType.Exp)
            return e

        def emit_ctx(h, e, psC, kt):
            for qs in range(2):
                nc.tensor.matmul(
                    psC[qs][0:65, :],
                    lhsT=vaug[:, kt, h, :],
                    rhs=e[:, ts(qs, 512)],
                    start=(kt == 0),
                    stop=(kt == NKT - 1),
                )

        def emit_norm_a(h, qh, psC):
            # Evacuate psC promptly: Z row to zsb, unnormalized ctx to cxu,
            # and launch the Z broadcast roundtrip right away (sync FIFO
            # orders the write before the broadcast read).
            blk = h * NQH + qh
            zsb = zp.tile([128, QH], F32, tag="z", name="zsb")
            cxu = cxp.tile([128, 2, 512], BF16, tag="cx", name="cxu")
            for qs in range(2):
                nc.vector.tensor_copy(out=zsb[64:65, ts(qs, 512)],
                                      in_=psC[qs][64:65, :])
                nc.vector.tensor_copy(out=cxu[0:64, qs, :], in_=psC[qs][0:64, :])
            nc.sync.dma_start(out=zscr[blk, :], in_=zsb[64:65, :])
            bc = bcp.tile([128, QH], F32, tag="bc", name="bc_t")
            seg = zscr[blk, :]
            nc.sync.dma_start(
                out=bc[0:64, :],
                in_=bass.AP(tensor=seg.tensor, offset=seg.offset,
                            ap=[[0, 64]] + list(seg.ap)))
            return bc, cxu

        def norm_stage2(h, qh, cxu, bc):
            rec = recp.tile([128, QH], F32, tag="rec", name="rec_t")
            nc.vector.reciprocal_approx_fast(out=rec[0:64, :], in_=bc[0:64, :])
            pend.append(lambda: norm_stage3(h, qh, cxu, rec))

        def norm_stage3(h, qh, cxu, rec):
            t = h // 2
            if qh not in stacks:
                stacks[qh] = stackp.tile([128, 2, QH], BF16, tag="stack",
                                         name="stack_t")
            if h % 2 == 0:
                for qs in range(2):
                    nc.vector.tensor_mul(stacks[qh][0:64, t, ts(qs, 512)],
                                         cxu[0:64, qs, :], rec[0:64, ts(qs, 512)])
            else:
                hb = hbp.tile([128, QH], BF16, tag="hb", name="hb_t")
                for qs in range(2):
                    nc.vector.tensor_mul(hb[0:64, ts(qs, 512)],
                                         cxu[0:64, qs, :], rec[0:64, ts(qs, 512)])
                nc.sync.dma_start(out=stacks[qh][ds(64, 64), t, :],
                                  in_=hb[0:64, :])
            if h == HLOC - 1:
                wo_ready.add(qh)

        def emit_wo_pair(qh, qt, od, big_psum=None):
            stack_t = stacks[qh]
            if big_psum is None:
                pw = pp_sh.tile([128, 512], F32, tag="sh", name="pw")
            else:
                pw = big_psum
            for t in range(2):
                nc.tensor.matmul(
                    pw,
                    lhsT=stack_t[:, t, ts(qt, 128)],
                    rhs=wo_sb[:, t, ts(od, 512)],
                    start=(t == 0), stop=(t == 1),
                )
            ob = obp.tile([128, 512], BF16, tag="ob", name="ob_t")
            nc.vector.tensor_copy(out=ob, in_=pw)
            nc.gpsimd.dma_start(
                out=outp[ds(qh * QH + qt * 128, 128), ts(od, 512)], in_=ob)

        def emit_wo_qt_tail(qh, qt, ps, use_scalar):
            # tail form: both od halves into one big psum tile, one wide
            # evacuation (on scalar when it has gone idle), one row store
            stack_t = stacks[qh]
            for od in range(2):
                for t in range(2):
                    nc.tensor.matmul(
                        ps[:, ts(od, 512)],
                        lhsT=stack_t[:, t, ts(qt, 128)],
                        rhs=wo_sb[:, t, ts(od, 512)],
                        start=(t == 0), stop=(t == 1),
                    )
            ob = obp.tile([128, QH], BF16, tag="obw", name="obw_t", bufs=4)
            if use_scalar:
                nc.scalar.copy(ob, ps)
            else:
                nc.vector.tensor_copy(out=ob, in_=ps)
            nc.gpsimd.dma_start(
                out=outp[ds(qh * QH + qt * 128, 128), :], in_=ob)

        def drain_one(allow_wo=True):
            # pend stages cost no PE time — flush them all; wo pairs cost
            # ~426ns of PE, at most one per slot
            while pend:
                pend.pop(0)()
            if allow_wo and pend_wo and pend_wo[0][0] in wo_ready:
                qh_, qt_, od_ = pend_wo.pop(0)
                emit_wo_pair(qh_, qt_, od_)

        # --- pipelined attention: scores stream runs CTX_LAG k-tiles ahead
        # of the ctx stream; V projection rides along during block 0 ---
        CTX_LAG = 10
        blocks = [(qh, h) for qh in range(NQH) for h in range(HLOC)]
        ctx_q = []       # (block idx, kt, e tile)
        psC_of = {}      # block idx -> psum chunk pair

        def pop_ctx():
            bi, kt, e = ctx_q.pop(0)
            qh, h = blocks[bi]
            if bi == 0:
                # V projection for this token tile (all 4 heads), placed just
                # ahead of its first consumer so late xv DMAs don't stall PE
                psv = pp_sh.tile([128, HLOC, DH], F32, tag="sh", name="psv")
                for c in range(NDCH):
                    nc.tensor.matmul(
                        psv,
                        lhsT=xtiles[("v", c, kt // 8)][:, ts(kt % 8, 128)],
                        rhs=w_sb["v"][:, c, :],
                        start=(c == 0),
                        stop=(c == NDCH - 1),
                    )
                nc.vector.tensor_tensor(
                    out=vaug[:, kt, :, 0:64],
                    in0=psv,
                    in1=bvbc,
                    op=mybir.AluOpType.add,
                )
            if bi not in psC_of:
                psC_of[bi] = [pp_c.tile([128, 512], F32, tag="c", name=f"psC{qs}")
                              for qs in range(2)]
            emit_ctx(h, e, psC_of[bi], kt)
            if kt == NKT - 1:
                psC = psC_of.pop(bi)
                bc, cxu = emit_norm_a(h, qh, psC)
                pend.append(lambda h=h, qh=qh, cxu=cxu, bc=bc:
                            norm_stage2(h, qh, cxu, bc))
                if h == HLOC - 1:
                    for qt in range(QH // 128):
                        for od in range(2):
                            pend_wo.append((qh, qt, od))

        for bi, (qh, h) in enumerate(blocks):
            for kt in range(NKT):
                e = emit_scores(h, qh, kt)
                ctx_q.append((bi, kt, e))
                if len(ctx_q) > CTX_LAG:
                    pop_ctx()
                if pend_pe and kt % 4 == 1:
                    pend_pe.pop(0)()
                drain_one(allow_wo=(kt % 2 == 0))

        # tail: drain remaining ctx, then norm stages, then wo at qt
        # granularity with psum/evacuation spread across engines
        while ctx_q:
            pop_ctx()
            drain_one(allow_wo=False)
        while pend:
            pend.pop(0)()
        if pend_wo and pend_wo[0][2] == 1:
            # odd leftover: its od==0 partner already drained in-loop
            qh_, qt_, od_ = pend_wo.pop(0)
            ps = pp_s.tile([128, QH], F32, tag="s", name="ps_tail")
            emit_wo_pair(qh_, qt_, od_, big_psum=ps[:, 512:1024])
        qts = [(qh_, qt_) for qh_, qt_, od_ in pend_wo if od_ == 0]
        for i, (qh_, qt_) in enumerate(qts):
            ps = pp_s.tile([128, QH], F32, tag="s", name="ps_tail")
            emit_wo_qt_tail(qh_, qt_, ps, use_scalar=(i % 2 == 0))

    nc.compile()
    return nc


_NC = None


def _get_nc():
    global _NC
    if _NC is None:
        _NC = _build_program()
    return _NC


def _host_prep(query, key, value, Wq, bq, Wk, bk, Wv, bv, Wo, bo):
    bf16 = ml_dtypes.bfloat16
    f32 = np.float32
    q = np.asarray(query, f32)
    k = np.asarray(key, f32)
    v = np.asarray(value, f32)
    Wq = np.asarray(Wq, f32)
    Wk = np.asarray(Wk, f32)
    Wv = np.asarray(Wv, f32)
    Wo = np.asarray(Wo, f32)
    bq = np.asarray(bq, f32)
    bk = np.asarray(bk, f32)
    bv = np.asarray(bv, f32)

    scale = np.float32(1.0 / np.sqrt(DH))
    xqT = np.ascontiguousarray(q.transpose(0, 2, 1)).astype(bf16)
    xkT = np.ascontiguousarray(k.transpose(0, 2, 1)).astype(bf16)
    xvT = np.ascontiguousarray(v.transpose(0, 2, 1)).astype(bf16)

    in_maps = []
    for c in range(NCORES):
        b = c // 4
        g = c % 4
        sl = slice(g * DLOC, (g + 1) * DLOC)
        in_maps.append({
            "xqT": xqT[b], "xkT": xkT[b], "xvT": xvT[b],
            "wq": np.ascontiguousarray(Wq[:, sl] * scale).astype(bf16),
            "wk": np.ascontiguousarray(Wk[:, sl]).astype(bf16),
            "wv": np.ascontiguousarray(Wv[:, sl]).astype(bf16),
            "bq": np.ascontiguousarray((bq[sl] * scale).reshape(DLOC, 1)),
            "bk": np.ascontiguousarray(bk[sl].reshape(DLOC, 1)),
            "bv": np.ascontiguousarray(bv[sl].reshape(1, DLOC)),
            "wo": np.ascontiguousarray(Wo[sl, :]).astype(bf16),
        })
    return in_maps


def _run(in_maps, trace=False):
    nc = _get_nc()
    return run_bass_kernel_spmd(nc, in_maps, list(range(NCORES)), trace=trace)


def kernel(query, key, value, Wq, bq, Wk, bk, Wv, bv, Wo, bo):
    in_maps = _host_prep(query, key, value, Wq, bq, Wk, bk, Wv, bv, Wo, bo)
    res = _run(in_maps)
    out = np.zeros((B, N, D), np.float32)
    for c in range(NCORES):
        out[c // 4] += np.asarray(res.results[c]["outp"], np.float32)
    out += np.asarray(bo, np.float32)[None, None, :]
    return out


# revision 25
# speedup vs baseline: 1.1550x; 1.1550x over previous
"""Multi-head attention (B=2, N=2048, D=1024, H=16, dh=64) on 8 TRN2 cores.

Sharding: (batch x head-group) -- core c handles batch c//4 and heads
[4*(c%4), 4*(c%4)+4) (256 local dims). Each core computes its heads'
Q/K/V projections, attention, and a partial output projection; the host
sums 4 partials per batch and adds bo. Halves per-core input DMA vs
head-only sharding (each core loads only its batch's activations).

Per-core design notes (PE kept continuously busy to hold the HAM clock
gate at 2.4 GHz; exp on the scalar engine is the second-longest stream
and runs as [128,1024] tiles to amortize per-instruction overhead):
  - X^T [D, N] supplied by host; Q^T/K^T computed with W stationary
    ([dl, tok], dl on partitions; bias via per-partition tensor_scalar).
  - V computed directly as [tok, dl] (x^T chunks stationary) into
    vaug = [V_h | ones] per head; ones column yields the softmax
    denominator Z for free during ctx accumulation.
  - scoresT[k, q] per (head, kt): two 512-col matmuls into a 2-bank
    [128,1024] f32 PSUM tile; one exp per tile.
  - ctxT[dv, q] accumulated over kt in [128,512] PSUM chunks; chunks
    are copied (unnormalized) to SBUF right away so the PSUM bank ring
    never stalls the PE at block boundaries.
  - normalization: Z row -> DRAM roundtrip partition-broadcast ->
    reciprocal_approx_fast -> multiply (engines cannot replicate
    across partitions; DMA can). Emitted as deferred stages inside the
    next block so latency hides behind matmuls.
  - output projection: out[q, od] = stack^T @ Wo in two K=128 pieces
    (dl-tiles), bf16 partials to HBM; host sums in f32.
  - DMA queues: sync = bulk loads, scalar = latency-critical Z/stack
    moves, gpsimd = output stores.
"""

import numpy as np
import ml_dtypes
from contextlib import ExitStack

import concourse.bass as bass
import concourse.tile as tile
from concourse import bacc, mybir
from concourse.bass import ts, ds
from concourse.bass_utils import run_bass_kernel_spmd

BF16 = mybir.dt.bfloat16
F32 = mybir.dt.float32

B = 2
N = 2048          # tokens per batch
D = 1024          # model dim
NCORES = 8
HLOC = 4          # heads per core
DLOC = 256        # local dims per core (4 heads x 64)
DH = 64
NKT = N // 128    # 16 k-tiles of 128
NDCH = D // 128   # 8 d-chunks
NQH = 2           # q halves of 1024
QH = 1024
NBLK = HLOC * NQH # 8 attention blocks per core


def _build_program():
    nc = bacc.Bacc("TRN2", target_bir_lowering=False, debug=False)

    xT = {}
    w = {}
    for t in ("q", "k", "v"):
        xT[t] = nc.dram_tensor(f"x{t}T", [D, N], BF16, kind="ExternalInput").ap()
        w[t] = nc.dram_tensor(f"w{t}", [D, DLOC], BF16, kind="ExternalInput").ap()
    bias = {}
    for t in ("q", "k"):
        bias[t] = nc.dram_tensor(f"b{t}", [DLOC, 1], F32, kind="ExternalInput").ap()
    bvd = nc.dram_tensor("bv", [1, DLOC], F32, kind="ExternalInput").ap()
    wo = nc.dram_tensor("wo", [DLOC, D], BF16, kind="ExternalInput").ap()
    outp = nc.dram_tensor("outp", [N, D], BF16, kind="ExternalOutput").ap()
    zscr = nc.dram_tensor("zscr", [NBLK, QH], F32).ap()

    with ExitStack() as ctx:
        tc = ctx.enter_context(tile.TileContext(nc))

        const = ctx.enter_context(tc.tile_pool(name="const", bufs=1))
        xqkp = ctx.enter_context(tc.tile_pool(name="xqk", bufs=32))
        xvp = ctx.enter_context(tc.tile_pool(name="xv", bufs=8))
        qkp = ctx.enter_context(tc.tile_pool(name="qk", bufs=1))
        vaugp = ctx.enter_context(tc.tile_pool(name="vaug", bufs=1))
        cxp = ctx.enter_context(tc.tile_pool(name="cxu", bufs=3))
        zp = ctx.enter_context(tc.tile_pool(name="zsb", bufs=2))
        bcp = ctx.enter_context(tc.tile_pool(name="bc", bufs=2))
        recp = ctx.enter_context(tc.tile_pool(name="rec", bufs=2))
        stackp = ctx.enter_context(tc.tile_pool(name="stack", bufs=2))
        hbp = ctx.enter_context(tc.tile_pool(name="hb", bufs=2))
        obp = ctx.enter_context(tc.tile_pool(name="ob", bufs=4))

        pp_s = ctx.enter_context(tc.tile_pool(name="pp_s", bufs=2, space="PSUM"))
        pp_c = ctx.enter_context(tc.tile_pool(name="pp_c", bufs=3, space="PSUM"))
        pp_sh = ctx.enter_context(tc.tile_pool(name="pp_sh", bufs=1, space="PSUM"))

        # ---- constants + x loads, ordered for just-in-time arrival ----
        w_sb = {}
        b_sb = {}
        xtiles = {}
        for t in ("q", "k"):
            w_sb[t] = const.tile([128, NDCH, DLOC], BF16, tag=f"w{t}", name=f"w{t}sb")
            nc.sync.dma_start(out=w_sb[t], in_=w[t].rearrange("(c p) m -> p c m", p=128))
            b_sb[t] = const.tile([128, 2, 1], F32, tag=f"b{t}", name=f"b{t}sb")
            nc.sync.dma_start(out=b_sb[t],
                              in_=bias[t].rearrange("(t p) o -> p t o", p=128))
            for hf in range(2):
                for c in range(NDCH):
                    xt_ = xqkp.tile([128, QH], BF16, tag="x", name="xt")
                    nc.sync.dma_start(out=xt_, in_=xT[t][ts(c, 128), ts(hf, QH)])
                    xtiles[(t, c, hf)] = xt_
        w_sb["v"] = const.tile([128, NDCH, DLOC], BF16, tag="wv", name="wvsb")
        nc.sync.dma_start(out=w_sb["v"], in_=w["v"].rearrange("(c p) m -> p c m", p=128))
        for c in range(NDCH):
            xt_ = xvp.tile([128, N], BF16, tag="xv", name="xvt")
            nc.sync.dma_start(out=xt_, in_=xT["v"][ts(c, 128), :])
            xtiles[("v", c)] = xt_
        bvbc = const.tile([128, HLOC, DH], F32, tag="bvbc")
        seg = bvd[0, :]
        nc.sync.dma_start(
            out=bvbc,
            in_=bass.AP(tensor=seg.tensor, offset=seg.offset,
                        ap=[[0, 128]] + list(seg.ap)))
        wo_sb = const.tile([128, 2, D], BF16, tag="wo")
        nc.sync.dma_start(out=wo_sb, in_=wo.rearrange("(t p) d -> p t d", p=128))

        # warm the exp table load off the critical path
        warm = const.tile([128, 1], BF16, tag="warm")
        nc.scalar.activation(warm, b_sb["q"][:, 0, :], mybir.ActivationFunctionType.Exp)

        # ---- PE warm-up burst: garbage matmuls during the DMA lead-in so
        # the HAM clock gate reaches 2.4 GHz before the projections start
        wmt = const.tile([128, 512], BF16, tag="wmt")
        nc.vector.memset(wmt, 0.0)
        for i in range(28):
            psW = pp_sh.tile([128, 512], F32, tag="sh", name="psW")
            nc.tensor.matmul(psW, lhsT=wmt[:, 0:128], rhs=wmt, start=True, stop=True)

        # ---- Q^T / K^T projections ([dl-tile, tok], W stationary) ----
        # Consume x half-tiles in DMA arrival order: per (t, half) both
        # dl-tiles' psum pairs live (exactly the 2-deep pp_s ring).
        qt_sb = qkp.tile([128, 2, N], BF16, tag="qt")
        kt_sb = qkp.tile([128, 2, N], BF16, tag="kt")
        dest = {"q": qt_sb, "k": kt_sb}
        for t in ("q", "k"):
            for hf in range(2):
                psD = [pp_s.tile([128, QH], F32, tag="s", name=f"psD{dl}")
                       for dl in range(2)]
                for c in range(NDCH):
                    for dl in range(2):
                        for sh in range(2):
                            nc.tensor.matmul(
                                psD[dl][:, ts(sh, 512)],
                                lhsT=w_sb[t][:, c, ds(dl * 128, 128)],
                                rhs=xtiles[(t, c, hf)][:, ts(sh, 512)],
                                start=(c == 0),
                                stop=(c == NDCH - 1),
                            )
                for dl in range(2):
                    for sh in range(2):
                        nc.vector.tensor_scalar_add(
                            dest[t][:, dl, ds(hf * QH + sh * 512, 512)],
                            psD[dl][:, ts(sh, 512)], b_sb[t][:, dl, :])

        # ---- attention ----
        # vaug: [tok-part, kt, h, 65] = [V_h | ones]
        vaug = vaugp.tile([128, NKT, HLOC, 65], BF16, tag="vaug")
        nc.vector.memset(vaug[:, :, :, 64:65], 1.0)

        stacks = {}      # qh -> stack tile [128, 2, QH]
        pend = []        # deferred work stages (one popped per drain slot)
        pend_wo = []     # deferred wo od-pairs
        wo_ready = set() # q-halves whose stack is fully written

        def emit_scores(h, qh, kt):
            lo = (h % 2) * 64
            t = h // 2
            psS = pp_s.tile([128, QH], F32, tag="s", name="psS")
            for half in range(2):
                nc.tensor.matmul(
                    psS[:, ts(half, 512)],
                    lhsT=kt_sb[ds(lo, 64), t, ts(kt, 128)],
                    rhs=qt_sb[ds(lo, 64), t, ds(qh * QH + half * 512, 512)],
                    start=True, stop=True,
                )
            # e tiles reuse the (dead) x half-tile ring: a deep buffer that
            # lets the ctx stream lag the scores stream by CTX_LAG k-tiles.
            e = xqkp.tile([128, QH], BF16, tag="x", name="e_t")
            nc.scalar.activation(e, psS, mybir.ActivationFunctionType.Exp)
            return e

        def emit_ctx(h, e, psC, kt):
            for qs in range(2):
                nc.tensor.matmul(
                    psC[qs][0:65, :],
                    lhsT=vaug[:, kt, h, :],
                    rhs=e[:, ts(qs, 512)],
                    start=(kt == 0),
                    stop=(kt == NKT - 1),
                )

        def emit_norm_a(h, qh, psC):
            # Evacuate psC promptly: Z row to zsb, unnormalized ctx to cxu,
            # and launch the Z broadcast roundtrip right away (sync FIFO
            # orders the write before the broadcast read).
            blk = h * NQH + qh
            zsb = zp.tile([128, QH], F32, tag="z", name="zsb")
            cxu = cxp.tile([128, 2, 512], BF16, tag="cx", name="cxu")
            for qs in range(2):
                nc.vector.tensor_copy(out=zsb[64:65, ts(qs, 512)],
                                      in_=psC[qs][64:65, :])
                nc.vector.tensor_copy(out=cxu[0:64, qs, :], in_=psC[qs][0:64, :])
            nc.sync.dma_start(out=zscr[blk, :], in_=zsb[64:65, :])
            bc = bcp.tile([128, QH], F32, tag="bc", name="bc_t")
            seg = zscr[blk, :]
            nc.sync.dma_start(
                out=bc[0:64, :],
                in_=bass.AP(tensor=seg.tensor, offset=seg.offset,
                            ap=[[0, 64]] + list(seg.ap)))
            return bc, cxu

        def norm_stage2(h, qh, cxu, bc):
            rec = recp.tile([128, QH], F32, tag="rec", name="rec_t")
            nc.vector.reciprocal_approx_fast(out=rec[0:64, :], in_=bc[0:64, :])
            pend.append(lambda: norm_stage3(h, qh, cxu, rec))

        def norm_stage3(h, qh, cxu, rec):
            t = h // 2
            if qh not in stacks:
                stacks[qh] = stackp.tile([128, 2, QH], BF16, tag="stack",
                                         name="stack_t")
            if h % 2 == 0:
                for qs in range(2):
                    nc.vector.tensor_mul(stacks[qh][0:64, t, ts(qs, 512)],
                                         cxu[0:64, qs, :], rec[0:64, ts(qs, 512)])
            else:
                hb = hbp.tile([128, QH], BF16, tag="hb", name="hb_t")
                for qs in range(2):
                    nc.vector.tensor_mul(hb[0:64, ts(qs, 512)],
                                         cxu[0:64, qs, :], rec[0:64, ts(qs, 512)])
                nc.sync.dma_start(out=stacks[qh][ds(64, 64), t, :],
                                  in_=hb[0:64, :])
            if h == HLOC - 1:
                wo_ready.add(qh)

        def emit_wo_pair(qh, qt, od, big_psum=None):
            stack_t = stacks[qh]
            if big_psum is None:
                pw = pp_sh.tile([128, 512], F32, tag="sh", name="pw")
            else:
                pw = big_psum
            for t in range(2):
                nc.tensor.matmul(
                    pw,
                    lhsT=stack_t[:, t, ts(qt, 128)],
                    rhs=wo_sb[:, t, ts(od, 512)],
                    start=(t == 0), stop=(t == 1),
                )
            ob = obp.tile([128, 512], BF16, tag="ob", name="ob_t")
            nc.vector.tensor_copy(out=ob, in_=pw)
            nc.gpsimd.dma_start(
                out=outp[ds(qh * QH + qt * 128, 128), ts(od, 512)], in_=ob)

        def emit_wo_qt_tail(qh, qt, ps, use_scalar):
            # tail form: both od halves into one big psum tile, one wide
            # evacuation (on scalar when it has gone idle), one row store
            stack_t = stacks[qh]
            for od in range(2):
                for t in range(2):
                    nc.tensor.matmul(
                        ps[:, ts(od, 512)],
                        lhsT=stack_t[:, t, ts(qt, 128)],
                        rhs=wo_sb[:, t, ts(od, 512)],
                        start=(t == 0), stop=(t == 1),
                    )
            ob = obp.tile([128, QH], BF16, tag="obw", name="obw_t", bufs=4)
            if use_scalar:
                nc.scalar.copy(ob, ps)
            else:
                nc.vector.tensor_copy(out=ob, in_=ps)
            nc.gpsimd.dma_start(
                out=outp[ds(qh * QH + qt * 128, 128), :], in_=ob)

        def drain_one(allow_wo=True):
            # pend stages cost no PE time — flush them all; wo pairs cost
            # ~426ns of PE, at most one per slot
            while pend:
                pend.pop(0)()
            if allow_wo and pend_wo and pend_wo[0][0] in wo_ready:
                qh_, qt_, od_ = pend_wo.pop(0)
                emit_wo_pair(qh_, qt_, od_)

        # --- pipelined attention: scores stream runs CTX_LAG k-tiles ahead
        # of the ctx stream; V projection rides along during block 0 ---
        CTX_LAG = 8
        blocks = [(qh, h) for qh in range(NQH) for h in range(HLOC)]
        ctx_q = []       # (block idx, kt, e tile)
        psC_of = {}      # block idx -> psum chunk pair

        def pop_ctx():
            bi, kt, e = ctx_q.pop(0)
            qh, h = blocks[bi]
            if bi == 0:
                # V projection for this token tile (all 4 heads), placed just
                # ahead of its first consumer so late xv DMAs don't stall PE
                psv = pp_sh.tile([128, HLOC, DH], F32, tag="sh", name="psv")
                for c in range(NDCH):
                    nc.tensor.matmul(
                        psv,
                        lhsT=xtiles[("v", c)][:, ts(kt, 128)],
                        rhs=w_sb["v"][:, c, :],
                        start=(c == 0),
                        stop=(c == NDCH - 1),
                    )
                nc.vector.tensor_tensor(
                    out=vaug[:, kt, :, 0:64],
                    in0=psv,
                    in1=bvbc,
                    op=mybir.AluOpType.add,
                )
            if bi not in psC_of:
                psC_of[bi] = [pp_c.tile([128, 512], F32, tag="c", name=f"psC{qs}")
                              for qs in range(2)]
            emit_ctx(h, e, psC_of[bi], kt)
            if kt == NKT - 1:
                psC = psC_of.pop(bi)
                bc, cxu = emit_norm_a(h, qh, psC)
                pend.append(lambda h=h, qh=qh, cxu=cxu, bc=bc:
                            norm_stage2(h, qh, cxu, bc))
                if h == HLOC - 1:
                    for qt in range(QH // 128):
                        for od in range(2):
                            pend_wo.append((qh, qt, od))

        for bi, (qh, h) in enumerate(blocks):
            for kt in range(NKT):
                e = emit_scores(h, qh, kt)
                ctx_q.append((bi, kt, e))
                if len(ctx_q) > CTX_LAG:
                    pop_ctx()
                drain_one(allow_wo=(kt % 2 == 0))

        # tail: drain remaining ctx, then norm stages, then wo at qt
        # granularity with psum/evacuation spread across engines
        while ctx_q:
            pop_ctx()
            drain_one(allow_wo=False)
        while pend:
            pend.pop(0)()
        if pend_wo and pend_wo[0][2] == 1:
            # odd leftover: its od==0 partner already drained in-loop
            qh_, qt_, od_ = pend_wo.pop(0)
            ps = pp_s.tile([128, QH], F32, tag="s", name="ps_tail")
            emit_wo_pair(qh_, qt_, od_, big_psum=ps[:, 512:1024])
        qts = [(qh_, qt_) for qh_, qt_, od_ in pend_wo if od_ == 0]
        for i, (qh_, qt_) in enumerate(qts):
            ps = pp_s.tile([128, QH], F32, tag="s", name="ps_tail")
            emit_wo_qt_tail(qh_, qt_, ps, use_scalar=(i % 2 == 0))

    nc.compile()
    return nc


_NC = None


def _get_nc():
    global _NC
    if _NC is None:
        _NC = _build_program()
    return _NC


def _host_prep(query, key, value, Wq, bq, Wk, bk, Wv, bv, Wo, bo):
    bf16 = ml_dtypes.bfloat16
    f32 = np.float32
    q = np.asarray(query, f32)
    k = np.asarray(key, f32)
    v = np.asarray(value, f32)
    Wq = np.asarray(Wq, f32)
    Wk = np.asarray(Wk, f32)
    Wv = np.asarray(Wv, f32)
    Wo = np.asarray(Wo, f32)
    bq = np.asarray(bq, f32)
    bk = np.asarray(bk, f32)
    bv = np.asarray(bv, f32)

    scale = np.float32(1.0 / np.sqrt(DH))
    xqT = np.ascontiguousarray(q.transpose(0, 2, 1)).astype(bf16)
    xkT = np.ascontiguousarray(k.transpose(0, 2, 1)).astype(bf16)
    xvT = np.ascontiguousarray(v.transpose(0, 2, 1)).astype(bf16)

    in_maps = []
    for c in range(NCORES):
        b = c // 4
        g = c % 4
        sl = slice(g * DLOC, (g + 1) * DLOC)
        in_maps.append({
            "xqT": xqT[b], "xkT": xkT[b], "xvT": xvT[b],
            "wq": np.ascontiguousarray(Wq[:, sl] * scale).astype(bf16),
            "wk": np.ascontiguousarray(Wk[:, sl]).astype(bf16),
            "wv": np.ascontiguousarray(Wv[:, sl]).astype(bf16),
            "bq": np.ascontiguousarray((bq[sl] * scale).reshape(DLOC, 1)),
            "bk": np.ascontiguousarray(bk[sl].reshape(DLOC, 1)),
            "bv": np.ascontiguousarray(bv[sl].reshape(1, DLOC)),
            "wo": np.ascontiguousarray(Wo[sl, :]).astype(bf16),
        })
    return in_maps


def _run(in_maps, trace=False):
    nc = _get_nc()
    return run_bass_kernel_spmd(nc, in_maps, list(range(NCORES)), trace=trace)


def kernel(query, key, value, Wq, bq, Wk, bk, Wv, bv, Wo, bo):
    in_maps = _host_prep(query, key, value, Wq, bq, Wk, bk, Wv, bv, Wo, bo)
    res = _run(in_maps)
    out = np.zeros((B, N, D), np.float32)
    for c in range(NCORES):
        out[c // 4] += np.asarray(res.results[c]["outp"], np.float32)
    out += np.asarray(bo, np.float32)[None, None, :]
    return out
